# revision 17
# baseline (speedup 1.0000x reference)
"""KoLeo-loss kernel for 8 Trainium2 NeuronCores.

Reference computation (for x of shape [B=16384, D=256] f32):
    xn   = x / ||x||_row                       (L2 row normalize)
    gram = xn @ xn.T
    min_dist_i = min_{j != i} sqrt(clip(2 - 2*gram_ij, 0))
    loss = -mean(log(min_dist + 1e-8))

Device strategy (one identical SPMD program on 8 cores):
  - Core c receives xr = roll(x, -c*2048, axis=0): its 2048 query rows are
    local rows 0..2047, and the self-match (diagonal) of local query m sits
    at local column m.  Row-max is permutation invariant, so rolling is free.
  - Phase A: load 128-row chunks, row-normalize in f32 (ACT square+accum,
    ACT sqrt, DVE reciprocal, DVE scale+cast to fp16), PE-transpose into a
    feature-major fp16 tile xT [128p(feature), 2(k), n_rows].
  - Phase B: for each 128-query chunk (stationary = slice of xT), stream all
    database columns through the PE in 512-col PSUM banks (K=256 as two
    accumulated passes).  Drain: ACT copies half the banks PSUM->SBUF f32;
    DVE tensor_tensor_reduce(max, max) consumes (psum bank, sbuf copy) pairs
    and maintains the running row max in a [128,1] accumulator.  Self-match
    is killed by adding -4 to the one 512-col bank holding the diagonal.
  - Output per core: gmax [128, 16] f32 (row-max of gram per query).
Host finishes: min_dist = sqrt(2-2*gmax), loss = -mean(log(min_dist+1e-8)).
"""

import sys

if "/opt/trn_rl_repo" not in sys.path:
    sys.path.insert(0, "/opt/trn_rl_repo")

import numpy as np

D = 256
P = 128
BANK = 512  # psum bank width in f32 elements
SPAN = 8  # psum banks in flight per span
B_FULL = 16384
N_CORES = 8
QPC = B_FULL // N_CORES  # queries per core


def make_dmask() -> np.ndarray:
    """dmask[p, t, j] = -4 where j == t*128+p else 0.

    Query chunk mc (local rows mc*128+p) has its self-match in bank mc//4
    at in-bank column (mc%4)*128 + p; tile t = mc%4 kills it.
    """
    dm = np.zeros((P, 4, BANK), dtype=np.float32)
    for t in range(4):
        for p in range(P):
            dm[p, t, t * P + p] = -4.0
    return dm


def build_nc(n_rows: int, n_q: int):
    import concourse.mybir as mybir
    import concourse.tile as tile
    from concourse import bacc
    from concourse.masks import make_identity

    dt = mybir.dt
    AF = mybir.ActivationFunctionType
    OP = mybir.AluOpType

    assert n_rows % (BANK * SPAN) == 0
    assert n_q % P == 0
    n_mc = n_q // P
    n_chunks = n_rows // P
    n_groups = n_chunks // 4
    n_banks = n_rows // BANK
    n_spans = n_banks // SPAN
    assert n_mc <= 4 * SPAN, "diag bank must land in span 0"

    nc = bacc.Bacc(None)
    x_in = nc.declare_dram_parameter("x", [n_rows, D], dt.float32, isOutput=False)
    dm_in = nc.declare_dram_parameter("dmask", [P, 4, BANK], dt.float32, isOutput=False)
    out_d = nc.declare_dram_parameter("gmax", [P, n_mc], dt.float32, isOutput=True)

    PAIR = 2 * BANK  # two psum banks per tile: fewer, bigger drain ops

    with tile.TileContext(nc) as tc:
        with (
            tc.tile_pool(name="persist", bufs=1) as persist,
            tc.tile_pool(name="ld", bufs=4) as ldp,
            tc.tile_pool(name="norm", bufs=6) as normp,
            tc.tile_pool(name="cp", bufs=8) as cpp,
            tc.tile_pool(name="mxp", bufs=2) as mxp,
            tc.tile_pool(name="ps", bufs=4, space="PSUM") as psp,
        ):
            xT = persist.tile([P, 2, n_rows], dt.float16)
            ident = persist.tile([P, P], dt.float16)
            make_identity(nc, ident)
            dmask = persist.tile([P, 4, BANK], dt.float32)
            nc.gpsimd.dma_start(out=dmask, in_=dm_in[:, :, :])
            gmax = persist.tile([P, n_mc], dt.float32)

            QUAD = 4 * BANK

            # One span: 8 banks as one QUAD (4 banks) + two PAIR psum tiles.
            # ACT copies QUAD+first PAIR to fp16 (6 banks, 2 ops); DVE eats
            # the last PAIR as a psum TT operand and folds everything into
            # the per-mc running max macc [128, 2048] fp16.
            def emit_span(mc, sp, macc):
                ptq = psp.tile([P, QUAD], dt.float32, tag="psq", bufs=1, name="ptq")
                pt1 = psp.tile([P, PAIR], dt.float32, tag="psp", bufs=2, name="pt1")
                pt2 = psp.tile([P, PAIR], dt.float32, tag="psp", bufs=2, name="pt2")
                for k in range(2):
                    lhs = xT[:, k, mc * P : (mc + 1) * P]
                    b0 = sp * SPAN
                    for h in range(4):
                        nc.tensor.matmul(
                            ptq[:, h * BANK : (h + 1) * BANK],
                            lhs,
                            xT[:, k, (b0 + h) * BANK : (b0 + h + 1) * BANK],
                            start=(k == 0),
                            stop=(k == 1),
                        )
                    for pi, pt in ((4, pt1), (6, pt2)):
                        for h in range(2):
                            nc.tensor.matmul(
                                pt[:, h * BANK : (h + 1) * BANK],
                                lhs,
                                xT[:, k, (b0 + pi + h) * BANK : (b0 + pi + h + 1) * BANK],
                                start=(k == 0),
                                stop=(k == 1),
                            )
                cq = cpp.tile([P, QUAD], dt.float16, tag="cq", bufs=3, name="cq")
                nc.scalar.copy(cq, ptq)
                cp1 = cpp.tile([P, PAIR], dt.float16, tag="cp1", bufs=3, name="cp1")
                nc.scalar.copy(cp1, pt1)
                if sp == 0:
                    db = mc // 4  # diagonal bank 0..3: always inside the QUAD
                    seg = cq[:, db * BANK : (db + 1) * BANK]
                    nc.vector.tensor_tensor(seg, seg, dmask[:, mc % 4, :], OP.add)
                a = cpp.tile([P, PAIR], dt.float16, tag="a", bufs=3, name="a")
                nc.vector.tensor_tensor(a, pt2, cp1, OP.max)
                if sp == 0:
                    nc.vector.tensor_copy(macc, cq)
                    nc.vector.tensor_tensor(
                        macc[:, 0:PAIR], a, macc[:, 0:PAIR], OP.max
                    )
                else:
                    nc.vector.tensor_tensor(macc, cq, macc, OP.max)
                    nc.vector.tensor_tensor(
                        macc[:, 0:PAIR], a, macc[:, 0:PAIR], OP.max
                    )

            def finish_mc(mc, macc):
                mh = cpp.tile([P, PAIR], dt.float16, tag="mh", bufs=2, name="mh")
                nc.vector.tensor_tensor(
                    mh, macc[:, 0:PAIR], macc[:, PAIR : 2 * PAIR], OP.max
                )
                nc.vector.tensor_reduce(
                    gmax[:, mc : mc + 1], mh, axis=mybir.AxisListType.X, op=OP.max
                )

            # ---------------- PE warmup burst (HAM un-throttle) -------------
            wps = psp.tile([P, P], dt.float32, tag="psp", bufs=2, name="warm")
            for _ in range(24):
                nc.tensor.matmul(wps, ident, ident, start=True, stop=True)

            # ---------------- Phase A: normalize + transpose ----------------
            # mc=0's spans are interleaved: span sp only needs banks
            # 8sp..8sp+7 = groups 8sp..8sp+7, so it runs as soon as they land.
            macc0 = mxp.tile([P, QUAD], dt.float16, tag="macc", name="macc0")
            xv = x_in[:, :].rearrange("(g c p) d -> g p c d", c=4, p=P)
            for g in range(n_groups):
                xa = ldp.tile([P, 4, D], dt.float32, tag="xa")
                nc.gpsimd.dma_start(out=xa, in_=xv[g])
                n2 = normp.tile([P, 4], dt.float32, tag="n2")
                sq = normp.tile([P, D], dt.float16, tag="sq")
                for c in range(4):
                    nc.scalar.activation(
                        out=sq,
                        in_=xa[:, c, :],
                        func=AF.Square,
                        accum_out=n2[:, c : c + 1],
                    )
                nrm = normp.tile([P, 4], dt.float32, tag="nrm")
                nc.scalar.sqrt(nrm, n2)
                rn = normp.tile([P, 4], dt.float32, tag="rn")
                nc.vector.reciprocal(rn, nrm)
                xn = normp.tile([P, 4, D], dt.float16, tag="xn")
                for c in range(4):
                    nc.vector.tensor_tensor(
                        xn[:, c, :],
                        xa[:, c, :],
                        rn[:, c : c + 1].to_broadcast([P, D]),
                        OP.mult,
                    )
                # Transpose via NORMAL matmul (out = xn_half.T @ I): faster
                # than transpose-mode and counts as PE activity for HAM.
                # Two chunks share one psum tile so the drain copy runs FD=512.
                for cc in range(2):
                    pst = psp.tile([P, 2, 2 * P], dt.float32, tag="psp", bufs=2)
                    for ci in range(2):
                        c = 2 * cc + ci
                        for k in range(2):
                            nc.tensor.matmul(
                                pst[:, k, ci * P : (ci + 1) * P],
                                xn[:, c, k * P : (k + 1) * P],
                                ident,
                                start=True,
                                stop=True,
                            )
                    s = g * 4 + 2 * cc
                    dst = xT[:, :, s * P : (s + 2) * P]
                    nc.vector.tensor_copy(dst, pst)
                if g % 8 == 7 and (g // 8) < n_spans:
                    emit_span(0, g // 8, macc0)
            finish_mc(0, macc0)

            # ---------------- Phase B: remaining query chunks ---------------
            for mc in range(1, n_mc):
                macc = mxp.tile([P, QUAD], dt.float16, tag="macc")
                for sp in range(n_spans):
                    emit_span(mc, sp, macc)
                finish_mc(mc, macc)

            nc.sync.dma_start(out=out_d[:, :], in_=gmax)

    nc.compile()
    return nc


_NC_CACHE = {}


def _get_nc(n_rows, n_q):
    key = (n_rows, n_q)
    if key not in _NC_CACHE:
        _NC_CACHE[key] = build_nc(n_rows, n_q)
    return _NC_CACHE[key]


LAST_RESULT = None  # BassKernelResults of the most recent run (for profiling)


def kernel(student_output: np.ndarray) -> np.ndarray:
    import os

    from concourse.bass_utils import run_bass_kernel_spmd

    global LAST_RESULT
    x = np.ascontiguousarray(student_output, dtype=np.float32)
    assert x.shape == (B_FULL, D)

    nc = _get_nc(B_FULL, QPC)
    dm = make_dmask()
    in_maps = [
        {"x": np.roll(x, -c * QPC, axis=0), "dmask": dm} for c in range(N_CORES)
    ]
    trace = bool(int(os.environ.get("KOLEO_TRACE", "0")))
    res = run_bass_kernel_spmd(
        nc, in_maps, core_ids=list(range(N_CORES)), trace=trace
    )
    LAST_RESULT = res

    gmax = np.empty(B_FULL, dtype=np.float32)
    for c in range(N_CORES):
        gm = res.results[c]["gmax"]  # [128, n_mc]
        gmax[c * QPC : (c + 1) * QPC] = gm.T.ravel()

    min_dist = np.sqrt(np.clip(2.0 - 2.0 * gmax.astype(np.float64), 0.0, None))
    loss = -np.mean(np.log(min_dist + 1e-8))
    return np.float32(loss)


if __name__ == "__main__":
    rng = np.random.default_rng(0)
    x = rng.standard_normal((B_FULL, D), dtype=np.float32)
    out = kernel(x)
    print("loss:", out)
